# revision 6
# baseline (speedup 1.0000x reference)
"""Trainium2 Bass kernel for nn_MCGRU — 4 pipelined batch-streams per core.

Math (per reference):
  demo = static @ demo_W.T + demo_b                      [bs, HID]
  xp   = x @ lab_W.T (+ lab_b folded into gate biases)   [bs, T, LAB]
  per-lab GRU over T steps, input size 1, hidden F=4:
    r = sig(gi_r+gh_r); z = sig(gi_z+gh_z); n = tanh(gi_n + r*gh_n)
    h' = z*h + (1-z)*n
  out = cat(demo, h_T.reshape) @ out_W.T + out_b         [bs, HID]

Design (per core: 128 batch rows, 4 independent streams of 32):
  - lab_W is composed into the per-gate input weights on the host
    (gi = (diag(Wih) . lab_W) @ x), so there is no xp phase; the scan's
    x-side matmuls read raw x slices straight from the DMA'd tiles.
  - Lab groups merged into the free dim: ops are [128=(lab-in-grp, f),
    2*32={g0 batch | g1 batch}].
  - State per stream is the PAIR (zh, aa) with h = zh - aa; recurrent
    matmuls consume zh and aa separately (negated weight copies), so the
    final h subtraction is off the per-step critical path; h is
    reconstructed off-chain only for zh = z*h and the output head.
  - Per (step, stream): one PSUM bank {prz | pnh | pni}; ONE sigmoid
    [128,128] for r+z; uu = tt + pni is accumulated via an identity
    matmul into pni's psum run; ONE tanh [128,64].
  - The 4 streams pipeline through PE -> ACT -> DVE with the serial
    per-step chain (~2.7us) as the binding constraint; ACT and DVE both
    run near saturation.
"""

import ml_dtypes
import numpy as np

BF16 = ml_dtypes.bfloat16
BS, T, LAB, DEMO, HID, F = 1024, 128, 64, 16, 32, 4
NCORES = 8
BSL = BS // NCORES   # 128 batch rows per core
NS = 4               # streams per core
SB = BSL // NS       # 64 batch rows per stream
G = 2                # lab groups
LPG = LAB // G       # 32 labs per group


def _pack_host(inputs):
    """Layout-only host packing of weights + per-core input shards."""
    x = np.asarray(inputs["x"], np.float32)
    static = np.asarray(inputs["static"], np.float32)
    demo_W = np.asarray(inputs["demo_W"], np.float32)
    demo_b = np.asarray(inputs["demo_b"], np.float32)
    lab_W = np.asarray(inputs["lab_W"], np.float32)
    lab_b = np.asarray(inputs["lab_b"], np.float32)
    Wih = np.asarray(inputs["Wih"], np.float32)
    bih = np.asarray(inputs["bih"], np.float32)
    Whh = np.asarray(inputs["Whh"], np.float32)
    bhh = np.asarray(inputs["bhh"], np.float32)
    out_W = np.asarray(inputs["out_W"], np.float32)
    out_b = np.asarray(inputs["out_b"], np.float32)

    shared = {}
    gates = {"r": slice(0, 4), "z": slice(4, 8), "n": slice(8, 12)}
    # Recurrent block-diagonal weights per group (lhsT [128,128]).
    for gate, sl in gates.items():
        for g in range(G):
            w = np.zeros((128, 128), np.float32)
            for i, l in enumerate(range(g * LPG, (g + 1) * LPG)):
                s = slice(i * 4, i * 4 + 4)
                w[s, s] = Whh[l, sl, :].T
            shared[f"wh{gate}{g}"] = w
            shared[f"wh{gate}n{g}"] = -w
        # Composed x-side weights: gi[(l,f), tb] = Wih[l,f] * xp[l, tb]
        #   = sum_l' (Wih[l,f] * lab_W[l, l']) x[l', tb]  -> dense [64, 128]
        for g in range(G):
            w = np.zeros((64, 128), np.float32)
            for i, l in enumerate(range(g * LPG, (g + 1) * LPG)):
                for f in range(4):
                    w[:, i * 4 + f] = Wih[l, sl][f] * lab_W[l, :]
            shared[f"wx{gate}{g}"] = w

    # Gate biases as lhsT rows (lab_b folded via Wih since x excludes it).
    def bias_vec(sl, g, with_ih, with_hh, with_fold):
        v = np.zeros(128, np.float32)
        for i, l in enumerate(range(g * LPG, (g + 1) * LPG)):
            b = np.zeros(4, np.float32)
            if with_ih:
                b += bih[l, sl]
            if with_hh:
                b += bhh[l, sl]
            if with_fold:
                b += Wih[l, sl] * lab_b[l]
            v[i * 4:i * 4 + 4] = b
        return v

    brz4 = np.zeros((4, 128), np.float32)
    brz4[0] = bias_vec(gates["r"], 0, True, True, True)
    brz4[1] = bias_vec(gates["r"], 1, True, True, True)
    brz4[2] = bias_vec(gates["z"], 0, True, True, True)
    brz4[3] = bias_vec(gates["z"], 1, True, True, True)
    shared["brz4"] = brz4
    bnh2 = np.zeros((2, 128), np.float32)
    bnh2[0] = bias_vec(gates["n"], 0, False, True, False)
    bnh2[1] = bias_vec(gates["n"], 1, False, True, False)
    shared["bnh2"] = bnh2
    bni2 = np.zeros((2, 128), np.float32)
    bni2[0] = bias_vec(gates["n"], 0, True, False, True)
    bni2[1] = bias_vec(gates["n"], 1, True, False, True)
    shared["bni2"] = bni2

    sel4 = np.zeros((4, 4 * SB), np.float32)
    for k in range(4):
        sel4[k, k * SB:(k + 1) * SB] = 1.0
    shared["sel4"] = sel4
    sel2 = np.zeros((2, 2 * SB), np.float32)
    sel2[0, :SB] = 1.0
    sel2[1, SB:] = 1.0
    shared["sel2"] = sel2
    shared["ident"] = np.eye(128, dtype=np.float32)

    # Output layer. feat index (l, f) -> col HID + l*4 + f of out_W.
    w_feat = out_W[:, HID:]  # [32, 256]
    for g in range(G):
        wo = np.zeros((128, HID), np.float32)
        for i, l in enumerate(range(g * LPG, (g + 1) * LPG)):
            wo[i * 4:(i + 1) * 4, :] = w_feat[:, l * 4:(l + 1) * 4].T
        shared[f"wout{g}"] = wo
    shared["woutd"] = np.ascontiguousarray(out_W[:, :HID].T)  # [32, 32]
    shared["woutb"] = out_b.reshape(1, HID).copy()            # [1, 32]
    wdemo = np.zeros((DEMO + 1, HID), np.float32)
    wdemo[0, :] = demo_b
    wdemo[1:, :] = demo_W.T
    shared["wdemo"] = wdemo

    # Per-core shards. xs [64, T*BSL], col = t*BSL + b.
    xT = np.ascontiguousarray(x.transpose(2, 1, 0))  # [LAB, T, BS]
    in_maps = []
    for c in range(NCORES):
        m = dict(shared)
        xc = xT[:, :, c * BSL:(c + 1) * BSL]  # [64, 128, 128]
        m["xs"] = np.ascontiguousarray(xc.reshape(LAB, T * BSL))
        st = np.ones((DEMO + 1, BSL), np.float32)
        st[1:, :] = static[c * BSL:(c + 1) * BSL, :].T
        m["statt"] = st
        in_maps.append(m)
    bf_names = {"sel4", "sel2", "ident", "woutd",
                "brz4", "bnh2", "bni2", "wout0", "wout1"}
    for gate in gates:
        for g in range(G):
            bf_names |= {f"wh{gate}{g}", f"wh{gate}n{g}", f"wx{gate}{g}"}
    for m in in_maps:
        for n in list(m):
            if n in bf_names or n == "xs":
                m[n] = m[n].astype(BF16)
    return in_maps


def _build_kernel():
    import concourse.bacc as bacc
    import concourse.tile as tile
    from concourse import mybir
    from concourse._compat import get_trn_type

    f32 = mybir.dt.float32
    bf16 = mybir.dt.bfloat16
    nc = bacc.Bacc(get_trn_type() or "TRN2", target_bir_lowering=False, debug=False)
    A = mybir.AluOpType
    Sig = mybir.ActivationFunctionType.Sigmoid
    Tanh = mybir.ActivationFunctionType.Tanh

    d_xs = nc.dram_tensor("xs", (LAB, T * BSL), bf16, kind="ExternalInput")
    d_st = nc.dram_tensor("statt", (DEMO + 1, BSL), f32, kind="ExternalInput")
    wshapes = {
        "sel4": (4, 4 * SB, bf16), "sel2": (2, 2 * SB, bf16),
        "ident": (128, 128, bf16),
        "brz4": (4, 128, bf16), "bnh2": (2, 128, bf16), "bni2": (2, 128, bf16),
        "woutd": (HID, HID, bf16), "woutb": (1, HID, f32),
        "wdemo": (DEMO + 1, HID, f32),
        "wout0": (128, HID, bf16), "wout1": (128, HID, bf16),
    }
    for gate in ("r", "z", "n"):
        for g in range(G):
            wshapes[f"wh{gate}{g}"] = (128, 128, bf16)
            wshapes[f"wh{gate}n{g}"] = (128, 128, bf16)
            wshapes[f"wx{gate}{g}"] = (64, 128, bf16)
    dws = {n: nc.dram_tensor(n, s[:2], s[2], kind="ExternalInput")
           for n, s in wshapes.items()}
    d_y = nc.dram_tensor("y", (HID, BSL), f32, kind="ExternalOutput")

    with tile.TileContext(nc) as tc:
        with (
            tc.tile_pool(name="const", bufs=1) as cpool,
            tc.tile_pool(name="xsb", bufs=1) as xsbp,
            tc.tile_pool(name="state", bufs=4) as spool,
            tc.tile_pool(name="work", bufs=6) as wpool,
        ):
            wt = {}
            for name in list(wshapes) + ["statt"]:
                dt_ = dws[name] if name != "statt" else d_st
                t_ = cpool.tile(list(dt_.shape), dt_.dtype, tag=name)
                nc.gpsimd.dma_start(t_[:], dt_[:])
                wt[name] = t_

            # x tiles: rows = labs, col = t*BSL + b; quartered over t so the
            # scan can start as soon as the first chunk lands.
            QT = T // 4
            xs_q = [xsbp.tile([LAB, QT * BSL], bf16, tag=f"xs{q}",
                              name=f"xs{q}")
                    for q in range(4)]
            for q in range(4):
                half = T * BSL // 8
                for j in range(2):
                    cs = slice(j * half, (j + 1) * half)
                    nc.sync.dma_start(xs_q[q][:, cs],
                                      d_xs[:, q * 2 * half + j * half:
                                           q * 2 * half + (j + 1) * half])

            # ---- demo head (independent of scan) ----
            with tc.tile_pool(name="pd", bufs=1, space="PSUM") as pdpool:
                ps_d = pdpool.tile([HID, BSL], f32, tag="psd")
                nc.tensor.matmul(ps_d[:], wt["wdemo"][:], wt["statt"][:],
                                 start=True, stop=True)
                demo_sb = cpool.tile([HID, BSL], bf16, tag="demo_sb")
                nc.vector.tensor_copy(demo_sb[:], ps_d[:])

            # ---- GRU scan, 4 pipelined batch streams ----
            # State per stream is the pair (zh, aa) with h = zh - aa; the
            # recurrent matmuls consume zh and aa separately (negated weight
            # copies), so the h subtraction is off the critical path.
            zh_s, aa_s, hr_s = [], [], []
            for s in range(NS):
                z0 = spool.tile([128, 2 * SB], bf16, tag=f"zh{s}")
                nc.gpsimd.memset(z0[:], 0.0)
                a0 = spool.tile([128, 2 * SB], bf16, tag=f"aa{s}")
                nc.gpsimd.memset(a0[:], 0.0)
                zh_s.append(z0)
                aa_s.append(a0)
                hr_s.append(None)

            with tc.tile_pool(name="pg", bufs=2, space="PSUM") as pgp:
                pend = [None] * NS

                def head(s, t):
                    q, tq = divmod(t, T // 4)
                    c0 = tq * BSL + s * SB
                    xa = xs_q[q][:, c0:c0 + SB]

                    # One psum bank per (stream, step):
                    #   cols 0:128 prz | 128:192 pnh | 192:256 pni
                    # Each region gets TWO accumulation runs: an early one
                    # (bias + x + zh matmuls, ready mid-previous-step) and a
                    # short late continuation (aa matmuls / I@tt) so only the
                    # last-ready inputs sit on the critical path.
                    pg = pgp.tile([128, 8 * SB], f32, tag=f"pg{s}")
                    prz = pg[:, 0:4 * SB]
                    nc.tensor.matmul(prz, wt["brz4"][:], wt["sel4"][:],
                                     start=True, stop=False)
                    nc.tensor.matmul(prz[:, 0:SB], wt["wxr0"][:], xa,
                                     start=False, stop=False)
                    nc.tensor.matmul(prz[:, SB:2 * SB], wt["wxr1"][:], xa,
                                     start=False, stop=False)
                    nc.tensor.matmul(prz[:, 2 * SB:3 * SB], wt["wxz0"][:], xa,
                                     start=False, stop=False)
                    if t == 0:
                        nc.tensor.matmul(prz[:, 3 * SB:4 * SB], wt["wxz1"][:],
                                         xa, start=False, stop=True)
                    else:
                        nc.tensor.matmul(prz[:, 3 * SB:4 * SB], wt["wxz1"][:],
                                         xa, start=False, stop=False)
                        for i, (gate, g) in enumerate(
                                ((gt, gg) for gt in ("r", "z")
                                 for gg in range(G))):
                            cs = slice(i * SB, (i + 1) * SB)
                            nc.tensor.matmul(prz[:, cs], wt[f"wh{gate}{g}"][:],
                                             zh_s[s][:, g * SB:(g + 1) * SB],
                                             start=False, stop=(i == 3))
                        # late run: aa contributions only
                        for i, (gate, g) in enumerate(
                                ((gt, gg) for gt in ("r", "z")
                                 for gg in range(G))):
                            cs = slice(i * SB, (i + 1) * SB)
                            nc.tensor.matmul(prz[:, cs],
                                             wt[f"wh{gate}n{g}"][:],
                                             aa_s[s][:, g * SB:(g + 1) * SB],
                                             start=False, stop=(i == 3))

                    pnh = pg[:, 4 * SB:6 * SB]
                    if t == 0:
                        nc.tensor.matmul(pnh, wt["bnh2"][:], wt["sel2"][:],
                                         start=True, stop=True)
                    else:
                        nc.tensor.matmul(pnh, wt["bnh2"][:], wt["sel2"][:],
                                         start=True, stop=False)
                        for g in range(G):
                            cs = slice(g * SB, (g + 1) * SB)
                            nc.tensor.matmul(pnh[:, cs], wt[f"whn{g}"][:],
                                             zh_s[s][:, cs], start=False,
                                             stop=(g == G - 1))
                        for g in range(G):
                            cs = slice(g * SB, (g + 1) * SB)
                            nc.tensor.matmul(pnh[:, cs], wt[f"whnn{g}"][:],
                                             aa_s[s][:, cs], start=False,
                                             stop=(g == G - 1))
                    pni = pg[:, 6 * SB:8 * SB]
                    nc.tensor.matmul(pni, wt["bni2"][:], wt["sel2"][:],
                                     start=True, stop=False)
                    nc.tensor.matmul(pni[:, 0:SB], wt["wxn0"][:], xa,
                                     start=False, stop=False)
                    nc.tensor.matmul(pni[:, SB:2 * SB], wt["wxn1"][:], xa,
                                     start=False, stop=True)

                    # ONE sigmoid for r+z (psum -> sbuf bf16)
                    rz = wpool.tile([128, 4 * SB], bf16, tag=f"rz{s}")
                    nc.scalar.activation(rz[:], prz, Sig)
                    pend[s] = (pg, rz)

                def tail(s, t):
                    pg, rz = pend[s]
                    pnh = pg[:, 4 * SB:6 * SB]
                    pni = pg[:, 6 * SB:8 * SB]
                    r_ap = rz[:, 0:2 * SB]
                    z_ap = rz[:, 2 * SB:4 * SB]

                    # tt = r * pnh  (DVE; psum operand) -> sbuf bf16
                    tt = wpool.tile([128, 2 * SB], bf16, tag=f"tt{s}")
                    nc.vector.tensor_tensor(tt[:], r_ap, pnh, A.mult)
                    # uu = tt + pni via identity matmul, own late run
                    nc.tensor.matmul(pni, wt["ident"][:], tt[:],
                                     start=False, stop=True)
                    # nt = tanh(uu) (psum -> sbuf bf16)
                    nt = wpool.tile([128, 2 * SB], bf16, tag=f"nt{s}")
                    nc.scalar.activation(nt[:], pni, Tanh)

                    # new state pair: aa' = (z-1)*nt ; zh' = z*h
                    aan = spool.tile([128, 2 * SB], bf16, tag=f"aa{s}")
                    nc.vector.scalar_tensor_tensor(
                        aan[:], z_ap, 1.0, nt[:], A.subtract, A.mult)
                    zhn = spool.tile([128, 2 * SB], bf16, tag=f"zh{s}")
                    if t > 0:
                        nc.vector.tensor_tensor(zhn[:], z_ap, hr_s[s][:],
                                                A.mult)
                    else:
                        nc.gpsimd.memset(zhn[:], 0.0)
                    zh_s[s] = zhn
                    aa_s[s] = aan
                    # h for the NEXT step's zh, reconstructed off-chain
                    hr = wpool.tile([128, 2 * SB], bf16, tag=f"hr{s}")
                    nc.vector.tensor_tensor(hr[:], zhn[:], aan[:], A.subtract)
                    hr_s[s] = hr

                for t in range(T):
                    for s in range(NS):
                        head(s, t)
                        tail(s, t)

            # final h per stream for the output head
            h = []
            for s in range(NS):
                hf = wpool.tile([128, 2 * SB], bf16, tag=f"hf{s}")
                nc.vector.tensor_tensor(hf[:], zh_s[s][:], aa_s[s][:],
                                        A.subtract)
                h.append(hf)

            # ---- output head ----
            with tc.tile_pool(name="po", bufs=1, space="PSUM") as popool:
                ps_o = popool.tile([HID, BSL], f32, tag="pso")
                nc.tensor.matmul(ps_o[:], wt["woutd"][:], demo_sb[:],
                                 start=True, stop=False)
                nc.tensor.matmul(ps_o[:], wt["woutb"][:], wt["statt"][0:1, :],
                                 start=False, stop=False)
                for s in range(NS):
                    cs = slice(s * SB, (s + 1) * SB)
                    nc.tensor.matmul(ps_o[:, cs], wt["wout0"][:],
                                     h[s][:, 0:SB], start=False, stop=False)
                    nc.tensor.matmul(ps_o[:, cs], wt["wout1"][:],
                                     h[s][:, SB:2 * SB], start=False,
                                     stop=(s == NS - 1))
                y_sb = cpool.tile([HID, BSL], f32, tag="y_sb")
                nc.vector.tensor_copy(y_sb[:], ps_o[:])
                nc.sync.dma_start(d_y[:], y_sb[:])

    nc.compile()
    return nc


_NC_CACHE = None


def _get_nc():
    global _NC_CACHE
    if _NC_CACHE is None:
        _NC_CACHE = _build_kernel()
    return _NC_CACHE


def kernel(**inputs):
    from concourse import bass_utils

    in_maps = _pack_host(inputs)
    nc = _get_nc()
    res = bass_utils.run_bass_kernel_spmd(nc, in_maps, list(range(NCORES)))
    ys = [np.asarray(res.results[c]["y"]) for c in range(NCORES)]
    return np.ascontiguousarray(np.concatenate(ys, axis=1).T).astype(np.float32)


# revision 7
# speedup vs baseline: 1.0917x; 1.0917x over previous
"""Trainium2 Bass kernel for nn_MCGRU — 4 pipelined batch-streams per core.

Math (per reference):
  demo = static @ demo_W.T + demo_b                      [bs, HID]
  xp   = x @ lab_W.T (+ lab_b folded into gate biases)   [bs, T, LAB]
  per-lab GRU over T steps, input size 1, hidden F=4:
    r = sig(gi_r+gh_r); z = sig(gi_z+gh_z); n = tanh(gi_n + r*gh_n)
    h' = z*h + (1-z)*n
  out = cat(demo, h_T.reshape) @ out_W.T + out_b         [bs, HID]

Design (per core: 128 batch rows, 4 independent streams of 32):
  - lab_W is composed into the per-gate input weights on the host
    (gi = (diag(Wih) . lab_W) @ x), so there is no xp phase; the scan's
    x-side matmuls read raw x slices straight from the DMA'd tiles.
  - Lab groups merged into the free dim: ops are [128=(lab-in-grp, f),
    2*32={g0 batch | g1 batch}].
  - State per stream is the PAIR (zh, aa) with h = zh - aa; recurrent
    matmuls consume zh and aa separately (negated weight copies), so the
    final h subtraction is off the per-step critical path; h is
    reconstructed off-chain only for zh = z*h and the output head.
  - Per (step, stream): one PSUM bank {prz | pnh | pni}; ONE sigmoid
    [128,128] for r+z; uu = tt + pni is accumulated via an identity
    matmul into pni's psum run; ONE tanh [128,64].
  - The 4 streams pipeline through PE -> ACT -> DVE with the serial
    per-step chain (~2.7us) as the binding constraint; ACT and DVE both
    run near saturation.
"""

import ml_dtypes
import numpy as np

BF16 = ml_dtypes.bfloat16
BS, T, LAB, DEMO, HID, F = 1024, 128, 64, 16, 32, 4
NCORES = 8
BSL = BS // NCORES   # 128 batch rows per core
NS = 4               # streams per core
SB = BSL // NS       # 64 batch rows per stream
G = 2                # lab groups
LPG = LAB // G       # 32 labs per group


def _pack_host(inputs):
    """Layout-only host packing of weights + per-core input shards."""
    x = np.asarray(inputs["x"], np.float32)
    static = np.asarray(inputs["static"], np.float32)
    demo_W = np.asarray(inputs["demo_W"], np.float32)
    demo_b = np.asarray(inputs["demo_b"], np.float32)
    lab_W = np.asarray(inputs["lab_W"], np.float32)
    lab_b = np.asarray(inputs["lab_b"], np.float32)
    Wih = np.asarray(inputs["Wih"], np.float32)
    bih = np.asarray(inputs["bih"], np.float32)
    Whh = np.asarray(inputs["Whh"], np.float32)
    bhh = np.asarray(inputs["bhh"], np.float32)
    out_W = np.asarray(inputs["out_W"], np.float32)
    out_b = np.asarray(inputs["out_b"], np.float32)

    shared = {}
    gates = {"r": slice(0, 4), "z": slice(4, 8), "n": slice(8, 12)}
    # Recurrent block-diagonal weights per group (lhsT [128,128]).
    for gate, sl in gates.items():
        for g in range(G):
            w = np.zeros((128, 128), np.float32)
            for i, l in enumerate(range(g * LPG, (g + 1) * LPG)):
                s = slice(i * 4, i * 4 + 4)
                w[s, s] = Whh[l, sl, :].T
            shared[f"wh{gate}{g}"] = w
            shared[f"wh{gate}n{g}"] = -w
        # Composed x-side weights: gi[(l,f), tb] = Wih[l,f] * xp[l, tb]
        #   = sum_l' (Wih[l,f] * lab_W[l, l']) x[l', tb]  -> dense [64, 128]
        for g in range(G):
            w = np.zeros((64, 128), np.float32)
            for i, l in enumerate(range(g * LPG, (g + 1) * LPG)):
                for f in range(4):
                    w[:, i * 4 + f] = Wih[l, sl][f] * lab_W[l, :]
            shared[f"wx{gate}{g}"] = w

    # Gate biases as lhsT rows (lab_b folded via Wih since x excludes it).
    def bias_vec(sl, g, with_ih, with_hh, with_fold):
        v = np.zeros(128, np.float32)
        for i, l in enumerate(range(g * LPG, (g + 1) * LPG)):
            b = np.zeros(4, np.float32)
            if with_ih:
                b += bih[l, sl]
            if with_hh:
                b += bhh[l, sl]
            if with_fold:
                b += Wih[l, sl] * lab_b[l]
            v[i * 4:i * 4 + 4] = b
        return v

    brz4 = np.zeros((4, 128), np.float32)
    brz4[0] = bias_vec(gates["r"], 0, True, True, True)
    brz4[1] = bias_vec(gates["r"], 1, True, True, True)
    brz4[2] = bias_vec(gates["z"], 0, True, True, True)
    brz4[3] = bias_vec(gates["z"], 1, True, True, True)
    shared["brz4"] = brz4
    bnh2 = np.zeros((2, 128), np.float32)
    bnh2[0] = bias_vec(gates["n"], 0, False, True, False)
    bnh2[1] = bias_vec(gates["n"], 1, False, True, False)
    shared["bnh2"] = bnh2
    bni2 = np.zeros((2, 128), np.float32)
    bni2[0] = bias_vec(gates["n"], 0, True, False, True)
    bni2[1] = bias_vec(gates["n"], 1, True, False, True)
    shared["bni2"] = bni2

    sel4 = np.zeros((4, 4 * SB), np.float32)
    for k in range(4):
        sel4[k, k * SB:(k + 1) * SB] = 1.0
    shared["sel4"] = sel4
    sel2 = np.zeros((2, 2 * SB), np.float32)
    sel2[0, :SB] = 1.0
    sel2[1, SB:] = 1.0
    shared["sel2"] = sel2
    shared["ident"] = np.eye(128, dtype=np.float32)

    # Output layer. feat index (l, f) -> col HID + l*4 + f of out_W.
    w_feat = out_W[:, HID:]  # [32, 256]
    for g in range(G):
        wo = np.zeros((128, HID), np.float32)
        for i, l in enumerate(range(g * LPG, (g + 1) * LPG)):
            wo[i * 4:(i + 1) * 4, :] = w_feat[:, l * 4:(l + 1) * 4].T
        shared[f"wout{g}"] = wo
    shared["woutd"] = np.ascontiguousarray(out_W[:, :HID].T)  # [32, 32]
    shared["woutb"] = out_b.reshape(1, HID).copy()            # [1, 32]
    wdemo = np.zeros((DEMO + 1, HID), np.float32)
    wdemo[0, :] = demo_b
    wdemo[1:, :] = demo_W.T
    shared["wdemo"] = wdemo

    # Per-core shards. xs [64, T*BSL], col = t*BSL + b.
    xT = np.ascontiguousarray(x.transpose(2, 1, 0))  # [LAB, T, BS]
    in_maps = []
    for c in range(NCORES):
        m = dict(shared)
        xc = xT[:, :, c * BSL:(c + 1) * BSL]  # [64, 128, 128]
        m["xs"] = np.ascontiguousarray(xc.reshape(LAB, T * BSL))
        st = np.ones((DEMO + 1, BSL), np.float32)
        st[1:, :] = static[c * BSL:(c + 1) * BSL, :].T
        m["statt"] = st
        in_maps.append(m)
    bf_names = {"sel4", "sel2", "ident", "woutd",
                "brz4", "bnh2", "bni2", "wout0", "wout1"}
    for gate in gates:
        for g in range(G):
            bf_names |= {f"wh{gate}{g}", f"wh{gate}n{g}", f"wx{gate}{g}"}
    for m in in_maps:
        for n in list(m):
            if n in bf_names or n == "xs":
                m[n] = m[n].astype(BF16)
    return in_maps


def _build_kernel():
    import concourse.bacc as bacc
    import concourse.tile as tile
    from concourse import mybir
    from concourse._compat import get_trn_type

    f32 = mybir.dt.float32
    bf16 = mybir.dt.bfloat16
    nc = bacc.Bacc(get_trn_type() or "TRN2", target_bir_lowering=False, debug=False)
    A = mybir.AluOpType
    Sig = mybir.ActivationFunctionType.Sigmoid
    Tanh = mybir.ActivationFunctionType.Tanh

    d_xs = nc.dram_tensor("xs", (LAB, T * BSL), bf16, kind="ExternalInput")
    d_st = nc.dram_tensor("statt", (DEMO + 1, BSL), f32, kind="ExternalInput")
    wshapes = {
        "sel4": (4, 4 * SB, bf16), "sel2": (2, 2 * SB, bf16),
        "ident": (128, 128, bf16),
        "brz4": (4, 128, bf16), "bnh2": (2, 128, bf16), "bni2": (2, 128, bf16),
        "woutd": (HID, HID, bf16), "woutb": (1, HID, f32),
        "wdemo": (DEMO + 1, HID, f32),
        "wout0": (128, HID, bf16), "wout1": (128, HID, bf16),
    }
    for gate in ("r", "z", "n"):
        for g in range(G):
            wshapes[f"wh{gate}{g}"] = (128, 128, bf16)
            wshapes[f"wh{gate}n{g}"] = (128, 128, bf16)
            wshapes[f"wx{gate}{g}"] = (64, 128, bf16)
    dws = {n: nc.dram_tensor(n, s[:2], s[2], kind="ExternalInput")
           for n, s in wshapes.items()}
    d_y = nc.dram_tensor("y", (HID, BSL), f32, kind="ExternalOutput")

    with tile.TileContext(nc) as tc:
        with (
            tc.tile_pool(name="const", bufs=1) as cpool,
            tc.tile_pool(name="xsb", bufs=1) as xsbp,
            tc.tile_pool(name="state", bufs=4) as spool,
            tc.tile_pool(name="work", bufs=6) as wpool,
        ):
            wt = {}
            for name in list(wshapes) + ["statt"]:
                dt_ = dws[name] if name != "statt" else d_st
                t_ = cpool.tile(list(dt_.shape), dt_.dtype, tag=name)
                nc.gpsimd.dma_start(t_[:], dt_[:])
                wt[name] = t_

            # x tiles: rows = labs, col = t*BSL + b; quartered over t so the
            # scan can start as soon as the first chunk lands.
            QT = T // 4
            xs_q = [xsbp.tile([LAB, QT * BSL], bf16, tag=f"xs{q}",
                              name=f"xs{q}")
                    for q in range(4)]
            for q in range(4):
                half = T * BSL // 8
                for j in range(2):
                    cs = slice(j * half, (j + 1) * half)
                    nc.sync.dma_start(xs_q[q][:, cs],
                                      d_xs[:, q * 2 * half + j * half:
                                           q * 2 * half + (j + 1) * half])

            # ---- demo head (independent of scan) ----
            with tc.tile_pool(name="pd", bufs=1, space="PSUM") as pdpool:
                ps_d = pdpool.tile([HID, BSL], f32, tag="psd")
                nc.tensor.matmul(ps_d[:], wt["wdemo"][:], wt["statt"][:],
                                 start=True, stop=True)
                demo_sb = cpool.tile([HID, BSL], bf16, tag="demo_sb")
                nc.vector.tensor_copy(demo_sb[:], ps_d[:])

            # ---- GRU scan, 4 pipelined batch streams ----
            # State per stream is the pair (zh, aa) with h = zh - aa; the
            # recurrent matmuls consume zh and aa separately (negated weight
            # copies), so the h subtraction is off the critical path.
            zh_s, aa_s, hr_s = [], [], []
            for s in range(NS):
                z0 = spool.tile([128, 2 * SB], bf16, tag=f"zh{s}")
                nc.gpsimd.memset(z0[:], 0.0)
                a0 = spool.tile([128, 2 * SB], bf16, tag=f"aa{s}")
                nc.gpsimd.memset(a0[:], 0.0)
                zh_s.append(z0)
                aa_s.append(a0)
                hr_s.append(None)

            with tc.tile_pool(name="pg", bufs=2, space="PSUM") as pgp:
                pend = [None] * NS

                def head(s, t):
                    q, tq = divmod(t, T // 4)
                    c0 = tq * BSL + s * SB
                    xa = xs_q[q][:, c0:c0 + SB]

                    # One psum bank per (stream, step):
                    #   cols 0:128 prz | 128:192 pnh | 192:256 pni
                    # Each region gets TWO accumulation runs: an early one
                    # (bias + x + zh matmuls, ready mid-previous-step) and a
                    # short late continuation (aa matmuls / I@tt) so only the
                    # last-ready inputs sit on the critical path.
                    pg = pgp.tile([128, 8 * SB], f32, tag=f"pg{s}")
                    prz = pg[:, 0:4 * SB]
                    nc.tensor.matmul(prz, wt["brz4"][:], wt["sel4"][:],
                                     start=True, stop=False)
                    nc.tensor.matmul(prz[:, 0:SB], wt["wxr0"][:], xa,
                                     start=False, stop=False)
                    nc.tensor.matmul(prz[:, SB:2 * SB], wt["wxr1"][:], xa,
                                     start=False, stop=False)
                    nc.tensor.matmul(prz[:, 2 * SB:3 * SB], wt["wxz0"][:], xa,
                                     start=False, stop=False)
                    if t == 0:
                        nc.tensor.matmul(prz[:, 3 * SB:4 * SB], wt["wxz1"][:],
                                         xa, start=False, stop=True)
                    else:
                        nc.tensor.matmul(prz[:, 3 * SB:4 * SB], wt["wxz1"][:],
                                         xa, start=False, stop=True)
                        for i, (gate, g) in enumerate(
                                ((gt, gg) for gt in ("r", "z")
                                 for gg in range(G))):
                            cs = slice(i * SB, (i + 1) * SB)
                            nc.tensor.matmul(prz[:, cs], wt[f"wh{gate}{g}"][:],
                                             zh_s[s][:, g * SB:(g + 1) * SB],
                                             start=False, stop=(i == 3))
                        # late run: aa contributions only
                        for i, (gate, g) in enumerate(
                                ((gt, gg) for gt in ("r", "z")
                                 for gg in range(G))):
                            cs = slice(i * SB, (i + 1) * SB)
                            nc.tensor.matmul(prz[:, cs],
                                             wt[f"wh{gate}n{g}"][:],
                                             aa_s[s][:, g * SB:(g + 1) * SB],
                                             start=False, stop=(i == 3))

                    pnh = pg[:, 4 * SB:6 * SB]
                    if t == 0:
                        nc.tensor.matmul(pnh, wt["bnh2"][:], wt["sel2"][:],
                                         start=True, stop=True)
                    else:
                        nc.tensor.matmul(pnh, wt["bnh2"][:], wt["sel2"][:],
                                         start=True, stop=True)
                        for g in range(G):
                            cs = slice(g * SB, (g + 1) * SB)
                            nc.tensor.matmul(pnh[:, cs], wt[f"whn{g}"][:],
                                             zh_s[s][:, cs], start=False,
                                             stop=(g == G - 1))
                        for g in range(G):
                            cs = slice(g * SB, (g + 1) * SB)
                            nc.tensor.matmul(pnh[:, cs], wt[f"whnn{g}"][:],
                                             aa_s[s][:, cs], start=False,
                                             stop=(g == G - 1))
                    pni = pg[:, 6 * SB:8 * SB]
                    nc.tensor.matmul(pni, wt["bni2"][:], wt["sel2"][:],
                                     start=True, stop=False)
                    nc.tensor.matmul(pni[:, 0:SB], wt["wxn0"][:], xa,
                                     start=False, stop=False)
                    nc.tensor.matmul(pni[:, SB:2 * SB], wt["wxn1"][:], xa,
                                     start=False, stop=True)

                    # ONE sigmoid for r+z (psum -> sbuf bf16)
                    rz = wpool.tile([128, 4 * SB], bf16, tag=f"rz{s}")
                    nc.scalar.activation(rz[:], prz, Sig)
                    pend[s] = (pg, rz)

                def tail(s, t):
                    pg, rz = pend[s]
                    pnh = pg[:, 4 * SB:6 * SB]
                    pni = pg[:, 6 * SB:8 * SB]
                    r_ap = rz[:, 0:2 * SB]
                    z_ap = rz[:, 2 * SB:4 * SB]

                    # tt = r * pnh  (DVE; psum operand) -> sbuf bf16
                    tt = wpool.tile([128, 2 * SB], bf16, tag=f"tt{s}")
                    nc.vector.tensor_tensor(tt[:], r_ap, pnh, A.mult)
                    # uu = tt + pni via identity matmul, own late run
                    nc.tensor.matmul(pni, wt["ident"][:], tt[:],
                                     start=False, stop=True)
                    # nt = tanh(uu) (psum -> sbuf bf16)
                    nt = wpool.tile([128, 2 * SB], bf16, tag=f"nt{s}")
                    nc.scalar.activation(nt[:], pni, Tanh)

                    # new state pair: aa' = (z-1)*nt ; zh' = z*h
                    aan = spool.tile([128, 2 * SB], bf16, tag=f"aa{s}")
                    nc.vector.scalar_tensor_tensor(
                        aan[:], z_ap, 1.0, nt[:], A.subtract, A.mult)
                    zhn = spool.tile([128, 2 * SB], bf16, tag=f"zh{s}")
                    if t > 0:
                        nc.vector.tensor_tensor(zhn[:], z_ap, hr_s[s][:],
                                                A.mult)
                    else:
                        nc.gpsimd.memset(zhn[:], 0.0)
                    zh_s[s] = zhn
                    aa_s[s] = aan
                    # h for the NEXT step's zh, reconstructed off-chain
                    hr = wpool.tile([128, 2 * SB], bf16, tag=f"hr{s}")
                    nc.vector.tensor_tensor(hr[:], zhn[:], aan[:], A.subtract)
                    hr_s[s] = hr

                for t in range(T):
                    for s in range(NS):
                        head(s, t)
                        tail(s, t)

            # final h per stream for the output head
            h = []
            for s in range(NS):
                hf = wpool.tile([128, 2 * SB], bf16, tag=f"hf{s}")
                nc.vector.tensor_tensor(hf[:], zh_s[s][:], aa_s[s][:],
                                        A.subtract)
                h.append(hf)

            # ---- output head ----
            with tc.tile_pool(name="po", bufs=1, space="PSUM") as popool:
                ps_o = popool.tile([HID, BSL], f32, tag="pso")
                nc.tensor.matmul(ps_o[:], wt["woutd"][:], demo_sb[:],
                                 start=True, stop=False)
                nc.tensor.matmul(ps_o[:], wt["woutb"][:], wt["statt"][0:1, :],
                                 start=False, stop=False)
                for s in range(NS):
                    cs = slice(s * SB, (s + 1) * SB)
                    nc.tensor.matmul(ps_o[:, cs], wt["wout0"][:],
                                     h[s][:, 0:SB], start=False, stop=False)
                    nc.tensor.matmul(ps_o[:, cs], wt["wout1"][:],
                                     h[s][:, SB:2 * SB], start=False,
                                     stop=(s == NS - 1))
                y_sb = cpool.tile([HID, BSL], f32, tag="y_sb")
                nc.vector.tensor_copy(y_sb[:], ps_o[:])
                nc.sync.dma_start(d_y[:], y_sb[:])

    nc.compile()
    return nc


_NC_CACHE = None


def _get_nc():
    global _NC_CACHE
    if _NC_CACHE is None:
        _NC_CACHE = _build_kernel()
    return _NC_CACHE


def kernel(**inputs):
    from concourse import bass_utils

    in_maps = _pack_host(inputs)
    nc = _get_nc()
    res = bass_utils.run_bass_kernel_spmd(nc, in_maps, list(range(NCORES)))
    ys = [np.asarray(res.results[c]["y"]) for c in range(NCORES)]
    return np.ascontiguousarray(np.concatenate(ys, axis=1).T).astype(np.float32)
